# revision 23
# baseline (speedup 1.0000x reference)
"""DeformableConv1D Trainium2 kernel.

Math: the reference reduces to
    offset = conv1d(x, Wconv) + bconv
    m = mean(offset);  scale_k = relu(1 - |m + R_k|);  s = sum_k Wdef[k]*scale_k
    out = conv1d(s*x, Wconv) + bconv = s * conv_nobias(x) + bconv

mean(offset) only needs per-channel sums of x (windowed; the tiny edge
corrections use 8 rows per batch and are folded into a host-precomputed
constant), so the device does:

  Phase 1 (streaming x once): 2MB load tiles (natural layout, 32-row units
  per partition for 1KB DMA descriptor rows); 8 PE transposes per [128,1024]
  psum group; one interleaved-read copy lands each group contiguously in the
  resident transposed tensor xt[(s',c), block] (cast to fp16), accumulating
  per-(s',c) sums as an accum_out side effect of the copies.

  The per-core mean contribution is collapsed to one scalar with a dot-product
  matmul, AllReduced (4 bytes) across the 8 cores, and s is computed on
  device. The s-dependent ops are emitted late in each engine's program order
  so PE/DVE/ACT keep working through the collective's latency.

  Phase 2: polyphase conv: per 128-block output tile two fp16 matmuls
  (A-weights on the xt window, B-weights on the +1-shifted window) accumulate
  in PSUM; unscaled [128,512] copies drain PSUM into 2MB stages; each stage is
  scaled in place by s (the only ops gated on the AllReduce) and DMA'd out,
  trailing the copies by 3 stages.

Sharding: data-parallel over batch (2 batches per core x 8 cores).
bconv is all-zero in this problem; if not, it is added on the host.
"""

import numpy as np

import concourse.bacc as bacc
import concourse.bass as bass
import concourse.mybir as mybir
import concourse.tile as tile
from concourse.tile_rust import add_dep_helper
from concourse.bass_utils import run_bass_kernel_spmd

FP = mybir.dt.float32
FPR = mybir.dt.float32r
CONV_DT = mybir.dt.float16

N_CORES = 8
B_TOTAL = 16
T = 65536
C = 32
F = 32
K = 5

BPC = B_TOTAL // N_CORES      # batches per core
U = 32                        # sub-tiles ([128,128] fp32) per load tile
NT = T // (4 * 128 * U)       # load tiles per batch (4)
QB = T // 4                   # blocks per batch (padded output length)
OT = QB // 128                # output tiles per batch (128)
GRP = U // 4                  # psum groups per load tile (8)
ST = 32                       # out-tiles per staged store (2 MiB)
NST = OT // ST                # stages per batch (8)
GPST = ST // 4                # psum groups per stage (4)


def build_kernel():
    nc = bacc.Bacc(
        "TRN2",
        target_bir_lowering=False,
        debug=False,
        enable_asserts=False,
        num_devices=N_CORES,
    )
    x = nc.dram_tensor("x", [BPC, T, C], FP, kind="ExternalInput").ap()
    wa = nc.dram_tensor("wa", [128, 128], CONV_DT, kind="ExternalInput").ap()
    wb = nc.dram_tensor("wb", [128, 128], CONV_DT, kind="ExternalInput").ap()
    ident = nc.dram_tensor("ident", [128, 128], FP, kind="ExternalInput").ap()
    qcv = nc.dram_tensor("qcv", [128, 1], FP, kind="ExternalInput").ap()
    c1 = nc.dram_tensor("c1", [1, 1], FP, kind="ExternalInput").ap()
    taps = nc.dram_tensor("taps", [1, K], FP, kind="ExternalInput").ap()
    wdef = nc.dram_tensor("wdef", [1, K], FP, kind="ExternalInput").ap()
    out = nc.dram_tensor("out", [BPC, QB, 128], FP, kind="ExternalOutput").ap()

    # load tiles: batch g, tile tt, partition p holds 4 units of 32 rows
    x_v = x.rearrange("g (tt v p r) c -> g tt p v (r c)", v=4, p=128, r=32)
    # out viewed for staged stores: block q = t*128 + p
    out_v = out.rearrange("g (t p) f -> g p t f", p=128)

    with tile.TileContext(nc) as tc:
        with (
            tc.tile_pool(name="res", bufs=1) as res_pool,
            tc.tile_pool(name="xload", bufs=3) as xload_pool,
            tc.tile_pool(name="stage", bufs=4) as stage_pool,
            tc.tile_pool(name="consts", bufs=1) as cpool,
            tc.tile_pool(name="pst", bufs=2, space="PSUM") as pst_pool,
            tc.tile_pool(name="ps", bufs=3, space="PSUM") as ps_pool,
            tc.tile_pool(name="psmisc", bufs=1, space="PSUM") as psmisc_pool,
            tc.tile_pool(name="dram", bufs=1, space="DRAM") as dram_pool,
        ):
            # resident transposed x: column g*QB + q holds x[g, 4q:4q+4, :]
            # flattened as (s', c) on partitions.
            xt = res_pool.tile([128, BPC * QB], CONV_DT)

            identity = cpool.tile([128, 128], FP)
            nc.gpsimd.dma_start(identity[:], ident[:])
            wa_t = cpool.tile([128, 128], CONV_DT)
            nc.gpsimd.dma_start(wa_t[:], wa[:])
            wb_t = cpool.tile([128, 128], CONV_DT)
            nc.gpsimd.dma_start(wb_t[:], wb[:])
            qcv_t = cpool.tile([128, 1], FP)
            nc.gpsimd.dma_start(qcv_t[:], qcv[:])
            c1_t = cpool.tile([1, 1], FP)
            nc.gpsimd.dma_start(c1_t[:], c1[:])
            taps_t = cpool.tile([1, K], FP)
            nc.gpsimd.dma_start(taps_t[:], taps[:])
            wdef_t = cpool.tile([1, K], FP)
            nc.gpsimd.dma_start(wdef_t[:], wdef[:])

            ones_row = cpool.tile([1, 128], FP)
            nc.vector.memset(ones_row[:], 1.0)

            acc = cpool.tile([128, BPC * NT * 4], FP)
            nc.vector.memset(acc[:], 0.0)
            psmisc = psmisc_pool.tile([128, 1], FP)

            # ---- Phase 1: load, channel sums, transpose into xt ----
            ncopy = 0
            for g in range(BPC):
                for tt in range(NT):
                    xtile = xload_pool.tile([128, U * 128], FP)
                    nc.sync.dma_start(
                        xtile.rearrange("p (v rc) -> p v rc", v=4), x_v[g, tt]
                    )
                    for v in range(4):
                        # tiny fp16 matmul keeps the PE HAM warm (transposes
                        # don't count as PE activity for the clock gate)
                        nc.tensor.matmul(psmisc[:], wa_t[:], wb_t[:, 0:1])
                        pt = pst_pool.tile([128, 1024], FP, name="pt")
                        for j in range(8):
                            nc.tensor.transpose(
                                pt[:, j * 128 : (j + 1) * 128],
                                xtile[:, v * 1024 + j * 128 : v * 1024 + (j + 1) * 128],
                                identity[:],
                            )
                        base = g * QB + (tt * 4 + v) * 1024
                        src_ap = pt.rearrange("i (j p) -> i p j", j=8, p=128)
                        if v == 3:
                            nc.vector.tensor_reduce(
                                acc[:, ncopy : ncopy + 1],
                                pt[:],
                                axis=mybir.AxisListType.X,
                                op=mybir.AluOpType.add,
                            )
                            nc.vector.tensor_copy(xt[:, base : base + 1024], src_ap)
                        else:
                            nc.scalar.activation(
                                xt[:, base : base + 1024],
                                src_ap,
                                mybir.ActivationFunctionType.Copy,
                                accum_out=acc[:, ncopy : ncopy + 1],
                            )
                        ncopy += 1

            # per-(s',c) partial sums
            localsum = cpool.tile([128, 1], FP)
            nc.vector.tensor_reduce(
                localsum[:],
                acc[:],
                axis=mybir.AxisListType.X,
                op=mybir.AluOpType.add,
            )
            # local dot: m_local = sum_p localsum[p] * qcvec[p]  (PE, ungated)
            nc.tensor.matmul(psmisc[0:1, 0:1], localsum[:], qcv_t[:])
            mloc = cpool.tile([1, 1], FP)
            nc.vector.tensor_copy(mloc[:], psmisc[0:1, 0:1])

            # ---- AllReduce the scalar across cores ----
            ar_in = dram_pool.tile([1, 1], FP)
            ar_out = dram_pool.tile([1, 1], FP, addr_space="Shared")
            nc.sync.dma_start(ar_in[:], mloc[:])
            nc.gpsimd.collective_compute(
                "AllReduce",
                mybir.AluOpType.add,
                replica_groups=[list(range(N_CORES))],
                ins=[ar_in.opt()],
                outs=[ar_out.opt()],
            )
            mg = cpool.tile([1, 1], FP)
            nc.sync.dma_start(mg[:], ar_out[:])

            # ---- s computation, deferred: emitted after stage-2 copies so the
            # DVE/ACT instruction streams don't stall on the AllReduce before
            # draining early phase-2 PSUM groups ----
            s_b = cpool.tile([128, 1], FP)
            last_copy = {"v": None, "s": None}

            def emit_s_chain():
                m1 = cpool.tile([1, 1], FP)
                i0 = nc.vector.tensor_tensor(
                    m1[:], mg[:], c1_t[:], op=mybir.AluOpType.add
                )
                if last_copy["v"] is not None:
                    add_dep_helper(i0.ins, last_copy["v"].ins, sync=False,
                                   reason="s-chain after phase2 copies (DVE)")
                t1 = cpool.tile([1, K], FP)
                nc.vector.tensor_scalar_add(t1[:], taps_t[:], m1[:])
                t2 = cpool.tile([1, K], FP)
                i2 = nc.scalar.activation(t2[:], t1[:], mybir.ActivationFunctionType.Abs)
                if last_copy["s"] is not None:
                    add_dep_helper(i2.ins, last_copy["s"].ins, sync=False,
                                   reason="s-chain after phase2 copies (ACT)")
                t3 = cpool.tile([1, K], FP)
                nc.vector.tensor_scalar(
                    t3[:],
                    t2[:],
                    -1.0,
                    1.0,
                    op0=mybir.AluOpType.mult,
                    op1=mybir.AluOpType.add,
                )
                t4 = cpool.tile([1, K], FP)
                nc.vector.tensor_scalar_max(t4[:], t3[:], 0.0)
                t5 = cpool.tile([1, K], FP)
                nc.vector.tensor_tensor(
                    t5[:], t4[:], wdef_t[:], op=mybir.AluOpType.mult
                )
                s11 = cpool.tile([1, 1], FP)
                nc.vector.tensor_reduce(
                    s11[:], t5[:], axis=mybir.AxisListType.X, op=mybir.AluOpType.add
                )
                nc.gpsimd.partition_broadcast(s_b[:], s11[:])

            # ---- Phase 2: polyphase conv ----
            # scales (the only ops gated on the AllReduce) and stores trail the
            # copies by 2 stages so engines keep draining PSUM during the wait
            nco = 0
            pending = []

            def flush_stage():
                g0, st0, stg0 = pending.pop(0)
                if (g0 * NST + st0) % 2 == 0:
                    isc = nc.vector.tensor_scalar_mul(stg0[:], stg0[:], s_b[:])
                    if last_copy["v"] is not None:
                        add_dep_helper(isc.ins, last_copy["v"].ins, sync=False,
                                       reason="scale after later copies (DVE)")
                else:
                    isc = nc.scalar.activation(
                        stg0[:],
                        stg0[:],
                        mybir.ActivationFunctionType.Copy,
                        scale=s_b[:],
                    )
                    if last_copy["s"] is not None:
                        add_dep_helper(isc.ins, last_copy["s"].ins, sync=False,
                                       reason="scale after later copies (ACT)")
                deng = nc.sync if (g0 * NST + st0) % 2 == 0 else nc.scalar
                deng.dma_start(
                    out_v[g0, :, st0 * ST : (st0 + 1) * ST, :],
                    stg0.rearrange("p (t f) -> p t f", t=ST),
                )

            for g in range(BPC):
                for st in range(NST):
                    stg = stage_pool.tile([128, ST * 128], FP)
                    for gi in range(GPST):
                        po = ps_pool.tile([128, 512], FP, name="ps")
                        for i in range(4):
                            ot = st * ST + gi * 4 + i
                            col0 = g * QB + 128 * ot
                            sl = po[:, i * 128 : (i + 1) * 128]
                            nc.tensor.matmul(
                                sl, xt[:, col0 : col0 + 128], wa_t[:],
                                start=True, stop=False,
                            )
                            mb = 127 if (g == BPC - 1 and ot == OT - 1) else 128
                            nc.tensor.matmul(
                                po[0:mb, i * 128 : (i + 1) * 128],
                                xt[:, col0 + 1 : col0 + 1 + mb], wb_t[:],
                                start=False, stop=True,
                            )
                        dst = stg[:, gi * 512 : (gi + 1) * 512]
                        if nco % 2 == 0:
                            last_copy["v"] = nc.vector.tensor_copy(dst, po[:])
                        else:
                            last_copy["s"] = nc.scalar.activation(
                                dst, po[:], mybir.ActivationFunctionType.Copy
                            )
                        nco += 1
                    pending.append((g, st, stg))
                    if g == 0 and st == 3:
                        emit_s_chain()
                    if len(pending) > 3:
                        flush_stage()
            while pending:
                flush_stage()

    nc.compile()
    return nc


_NC_CACHE = None
_LAST_IN_MAPS = None


def _get_nc():
    global _NC_CACHE
    if _NC_CACHE is None:
        _NC_CACHE = build_kernel()
    return _NC_CACHE


def _host_consts(x, Wconv, bconv):
    Tout = T - K + 1
    Ntot = B_TOTAL * Tout * F
    Wsum = Wconv.sum(axis=2).astype(np.float64)  # (K, C)
    head = x[:, : K - 1, :].astype(np.float64).sum(axis=0)  # (4, C)
    tail = x[:, T - (K - 1) :, :].astype(np.float64).sum(axis=0)  # (4, C)
    pre = np.concatenate([np.zeros((1, C)), np.cumsum(head, axis=0)], axis=0)
    suf = np.concatenate([np.zeros((1, C)), np.cumsum(tail[::-1], axis=0)], axis=0)
    edge = 0.0
    for k in range(K):
        edge += (Wsum[k] * (pre[k] + suf[K - 1 - k])).sum()
    qc = (Wsum.sum(axis=0) / Ntot).astype(np.float32)
    qcvec = np.tile(qc, 4).reshape(128, 1)
    c1 = np.float32(-edge / Ntot + float(np.mean(bconv)))
    return qcvec, np.array([[c1]], np.float32)


def _build_ab(Wconv):
    A = np.zeros((128, 128), np.float32)
    B = np.zeros((128, 128), np.float32)
    for sp in range(4):
        for so in range(4):
            k = sp - so
            if 0 <= k < K:
                A[sp * 32 : (sp + 1) * 32, so * 32 : (so + 1) * 32] = Wconv[k]
            k2 = sp - so + 4
            if 0 <= k2 < K:
                B[sp * 32 : (sp + 1) * 32, so * 32 : (so + 1) * 32] = Wconv[k2]
    return A.astype(np.float16), B.astype(np.float16)


def kernel(x, Wconv, bconv, Wdef):
    x = np.ascontiguousarray(np.asarray(x, np.float32))
    Wconv = np.asarray(Wconv, np.float32)
    bconv = np.asarray(bconv, np.float32)
    Wdef = np.asarray(Wdef, np.float32)

    nc = _get_nc()
    A, B = _build_ab(Wconv)
    qcvec, c1 = _host_consts(x, Wconv, bconv)
    ident = np.eye(128, dtype=np.float32)
    taps = (np.arange(K, dtype=np.float32) - (K // 2)).reshape(1, K)
    wdef_r = Wdef.reshape(1, K).astype(np.float32)

    in_maps = []
    for core in range(N_CORES):
        in_maps.append(
            {
                "x": x[core * BPC : (core + 1) * BPC],
                "wa": A,
                "wb": B,
                "ident": ident,
                "qcv": qcvec,
                "c1": c1,
                "taps": taps,
                "wdef": wdef_r,
            }
        )
    global _LAST_IN_MAPS
    _LAST_IN_MAPS = in_maps
    res = run_bass_kernel_spmd(nc, in_maps, list(range(N_CORES)))
    Tout = T - K + 1
    out = np.empty((B_TOTAL, Tout, F), np.float32)
    for core in range(N_CORES):
        o = res.results[core]["out"].reshape(BPC, QB * 4, F)
        out[core * BPC : (core + 1) * BPC] = o[:, :Tout, :]
    if np.any(bconv):
        out += bconv.reshape(1, 1, F)
    return out


# revision 25
# speedup vs baseline: 1.1317x; 1.1317x over previous
"""DeformableConv1D Trainium2 kernel.

Math: the reference reduces to
    offset = conv1d(x, Wconv) + bconv
    m = mean(offset);  scale_k = relu(1 - |m + R_k|);  s = sum_k Wdef[k]*scale_k
    out = conv1d(s*x, Wconv) + bconv = s * conv_nobias(x) + bconv

mean(offset) only needs per-channel sums of x (windowed; the tiny edge
corrections use 8 rows per batch and are folded into a host-precomputed
constant), so the device does:

  Phase 1 (streaming x once): for each 2MB load tile (natural layout, 4 rows
  per partition) run a fp32r ones-matmul (channel sums, accumulated in PSUM)
  and 4 PE transposes per psum group; copy each [128,512] psum group into the
  resident transposed tensor xt[(s',c), block] (cast to fp16).

  AllReduce 512B of partial sums across the 8 cores; compute the scalar s on
  device.

  Phase 2: polyphase conv: per 128-block output tile two fp16 matmuls
  (A-weights on the xt window, B-weights on the +1-shifted window) accumulate
  in PSUM; unscaled [128,512] copies drain PSUM into 1MB stages; each stage is
  scaled in place by s (the only op gated on the AllReduce) and DMA'd out.

Sharding: data-parallel over batch (2 batches per core x 8 cores).
bconv is all-zero in this problem; if not, it is added on the host.
"""

import numpy as np

import concourse.bacc as bacc
import concourse.bass as bass
import concourse.mybir as mybir
import concourse.tile as tile
from concourse.tile_rust import add_dep_helper
from concourse.bass_utils import run_bass_kernel_spmd

FP = mybir.dt.float32
FPR = mybir.dt.float32r
CONV_DT = mybir.dt.float16

N_CORES = 8
B_TOTAL = 16
T = 65536
C = 32
F = 32
K = 5

BPC = B_TOTAL // N_CORES      # batches per core
U = 32                        # sub-tiles ([128,128] fp32) per load tile
NT = T // (4 * 128 * U)       # load tiles per batch (4)
QB = T // 4                   # blocks per batch (padded output length)
OT = QB // 128                # output tiles per batch (128)
GRP = U // 4                  # psum groups per load tile (8)
ST = 32                       # out-tiles per staged store (2 MiB)
NST = OT // ST                # stages per batch (8)
GPST = ST // 4                # psum groups per stage (4)


def build_kernel():
    nc = bacc.Bacc(
        "TRN2",
        target_bir_lowering=False,
        debug=False,
        enable_asserts=False,
        num_devices=N_CORES,
    )
    x = nc.dram_tensor("x", [BPC, T, C], FP, kind="ExternalInput").ap()
    wa = nc.dram_tensor("wa", [128, 128], CONV_DT, kind="ExternalInput").ap()
    wb = nc.dram_tensor("wb", [128, 128], CONV_DT, kind="ExternalInput").ap()
    ident = nc.dram_tensor("ident", [128, 128], FP, kind="ExternalInput").ap()
    qcv = nc.dram_tensor("qcv", [128, 1], FP, kind="ExternalInput").ap()
    c1 = nc.dram_tensor("c1", [1, 1], FP, kind="ExternalInput").ap()
    taps = nc.dram_tensor("taps", [1, K], FP, kind="ExternalInput").ap()
    wdef = nc.dram_tensor("wdef", [1, K], FP, kind="ExternalInput").ap()
    out = nc.dram_tensor("out", [BPC, QB, 128], FP, kind="ExternalOutput").ap()

    # load tiles: batch g, tile tt, partition p holds 4 units of 32 rows
    x_v = x.rearrange("g (tt v p r) c -> g tt p v (r c)", v=4, p=128, r=32)
    # out viewed for staged stores: block q = t*128 + p
    out_v = out.rearrange("g (t p) f -> g p t f", p=128)

    with tile.TileContext(nc) as tc:
        with (
            tc.tile_pool(name="res", bufs=1) as res_pool,
            tc.tile_pool(name="xload", bufs=3) as xload_pool,
            tc.tile_pool(name="stage", bufs=4) as stage_pool,
            tc.tile_pool(name="consts", bufs=1) as cpool,
            tc.tile_pool(name="pst", bufs=2, space="PSUM") as pst_pool,
            tc.tile_pool(name="ps", bufs=3, space="PSUM") as ps_pool,
            tc.tile_pool(name="psmisc", bufs=1, space="PSUM") as psmisc_pool,
            tc.tile_pool(name="dram", bufs=1, space="DRAM") as dram_pool,
        ):
            # resident transposed x: column g*QB + q holds x[g, 4q:4q+4, :]
            # flattened as (s', c) on partitions.
            xt = res_pool.tile([128, BPC * QB], CONV_DT)

            identity = cpool.tile([128, 128], FP)
            nc.gpsimd.dma_start(identity[:], ident[:])
            wa_t = cpool.tile([128, 128], CONV_DT)
            nc.gpsimd.dma_start(wa_t[:], wa[:])
            wb_t = cpool.tile([128, 128], CONV_DT)
            nc.gpsimd.dma_start(wb_t[:], wb[:])
            qcv_t = cpool.tile([128, 1], FP)
            nc.gpsimd.dma_start(qcv_t[:], qcv[:])
            c1_t = cpool.tile([1, 1], FP)
            nc.gpsimd.dma_start(c1_t[:], c1[:])
            taps_t = cpool.tile([1, K], FP)
            nc.gpsimd.dma_start(taps_t[:], taps[:])
            wdef_t = cpool.tile([1, K], FP)
            nc.gpsimd.dma_start(wdef_t[:], wdef[:])

            ones_row = cpool.tile([1, 128], FP)
            nc.vector.memset(ones_row[:], 1.0)

            acc = cpool.tile([128, BPC * NT * 4], FP)
            nc.vector.memset(acc[:], 0.0)
            psmisc = psmisc_pool.tile([128, 1], FP)

            # ---- Phase 1: load, channel sums, transpose into xt ----
            ncopy = 0
            for g in range(BPC):
                for tt in range(NT):
                    xtile = xload_pool.tile([128, U * 128], FP)
                    nc.sync.dma_start(
                        xtile.rearrange("p (v rc) -> p v rc", v=4), x_v[g, tt]
                    )
                    for v in range(4):
                        # tiny fp16 matmul keeps the PE HAM warm (transposes
                        # don't count as PE activity for the clock gate)
                        nc.tensor.matmul(psmisc[:], wa_t[:], wb_t[:, 0:1])
                        pt = pst_pool.tile([128, 1024], FP, name="pt")
                        for j in range(8):
                            nc.tensor.transpose(
                                pt[:, j * 128 : (j + 1) * 128],
                                xtile[:, v * 1024 + j * 128 : v * 1024 + (j + 1) * 128],
                                identity[:],
                            )
                        base = g * QB + (tt * 4 + v) * 1024
                        src_ap = pt.rearrange("i (j p) -> i p j", j=8, p=128)
                        if v == 3:
                            nc.vector.tensor_reduce(
                                acc[:, ncopy : ncopy + 1],
                                pt[:],
                                axis=mybir.AxisListType.X,
                                op=mybir.AluOpType.add,
                            )
                            nc.vector.tensor_copy(xt[:, base : base + 1024], src_ap)
                        else:
                            nc.scalar.activation(
                                xt[:, base : base + 1024],
                                src_ap,
                                mybir.ActivationFunctionType.Copy,
                                accum_out=acc[:, ncopy : ncopy + 1],
                            )
                        ncopy += 1

            # per-(s',c) partial sums
            localsum = cpool.tile([128, 1], FP)
            nc.vector.tensor_reduce(
                localsum[:],
                acc[:],
                axis=mybir.AxisListType.X,
                op=mybir.AluOpType.add,
            )
            # local dot: m_local = sum_p localsum[p] * qcvec[p]  (PE, ungated)
            nc.tensor.matmul(psmisc[0:1, 0:1], localsum[:], qcv_t[:])
            mloc = cpool.tile([1, 1], FP)
            nc.vector.tensor_copy(mloc[:], psmisc[0:1, 0:1])

            # ---- AllReduce the scalar across cores ----
            ar_in = dram_pool.tile([1, 1], FP)
            ar_out = dram_pool.tile([1, 1], FP, addr_space="Shared")
            nc.gpsimd.dma_start(ar_in[:], mloc[:])
            nc.gpsimd.collective_compute(
                "AllReduce",
                mybir.AluOpType.add,
                replica_groups=[list(range(N_CORES))],
                ins=[ar_in.opt()],
                outs=[ar_out.opt()],
            )
            mg = cpool.tile([1, 1], FP)
            nc.gpsimd.dma_start(mg[:], ar_out[:])

            # ---- s computation, deferred: emitted after stage-2 copies so the
            # DVE/ACT instruction streams don't stall on the AllReduce before
            # draining early phase-2 PSUM groups ----
            s_b = cpool.tile([128, 1], FP)
            last_copy = {"v": None, "s": None}

            def emit_s_chain():
                # pin the two s-chain entry ops after stage-3's copies on their
                # engines: the scheduler's cost model underestimates the
                # AllReduce, and without these edges it parks the DVE/ACT
                # streams on gated ops while PSUM-draining copies could run
                m1 = cpool.tile([1, 1], FP)
                i0 = nc.vector.tensor_tensor(
                    m1[:], mg[:], c1_t[:], op=mybir.AluOpType.add
                )
                if last_copy["v"] is not None:
                    add_dep_helper(i0.ins, last_copy["v"].ins, sync=False,
                                   reason="s-chain after gap copies (DVE)")
                t1 = cpool.tile([1, K], FP)
                nc.vector.tensor_scalar_add(t1[:], taps_t[:], m1[:])
                t2 = cpool.tile([1, K], FP)
                i2 = nc.scalar.activation(t2[:], t1[:], mybir.ActivationFunctionType.Abs)
                if last_copy["s"] is not None:
                    add_dep_helper(i2.ins, last_copy["s"].ins, sync=False,
                                   reason="s-chain after gap copies (ACT)")
                t3 = cpool.tile([1, K], FP)
                nc.vector.tensor_scalar(
                    t3[:],
                    t2[:],
                    -1.0,
                    1.0,
                    op0=mybir.AluOpType.mult,
                    op1=mybir.AluOpType.add,
                )
                t4 = cpool.tile([1, K], FP)
                nc.vector.tensor_scalar_max(t4[:], t3[:], 0.0)
                t5 = cpool.tile([1, K], FP)
                nc.vector.tensor_tensor(
                    t5[:], t4[:], wdef_t[:], op=mybir.AluOpType.mult
                )
                s11 = cpool.tile([1, 1], FP)
                nc.vector.tensor_reduce(
                    s11[:], t5[:], axis=mybir.AxisListType.X, op=mybir.AluOpType.add
                )
                nc.gpsimd.partition_broadcast(s_b[:], s11[:])

            # ---- Phase 2: polyphase conv ----
            # scales (the only ops gated on the AllReduce) and stores trail the
            # copies by 2 stages so engines keep draining PSUM during the wait
            nco = 0
            pending = []

            def flush_stage():
                g0, st0, stg0 = pending.pop(0)
                if (g0 * NST + st0) % 2 == 0:
                    nc.vector.tensor_scalar_mul(stg0[:], stg0[:], s_b[:])
                else:
                    nc.scalar.activation(
                        stg0[:],
                        stg0[:],
                        mybir.ActivationFunctionType.Copy,
                        scale=s_b[:],
                    )
                nc.sync.dma_start(
                    out_v[g0, :, st0 * ST : (st0 + 1) * ST, :],
                    stg0.rearrange("p (t f) -> p t f", t=ST),
                )

            for g in range(BPC):
                for st in range(NST):
                    stg = stage_pool.tile([128, ST * 128], FP)
                    for gi in range(GPST):
                        po = ps_pool.tile([128, 512], FP, name="ps")
                        for i in range(4):
                            ot = st * ST + gi * 4 + i
                            col0 = g * QB + 128 * ot
                            sl = po[:, i * 128 : (i + 1) * 128]
                            nc.tensor.matmul(
                                sl, xt[:, col0 : col0 + 128], wa_t[:],
                                start=True, stop=False,
                            )
                            mb = 127 if (g == BPC - 1 and ot == OT - 1) else 128
                            nc.tensor.matmul(
                                po[0:mb, i * 128 : (i + 1) * 128],
                                xt[:, col0 + 1 : col0 + 1 + mb], wb_t[:],
                                start=False, stop=True,
                            )
                        dst = stg[:, gi * 512 : (gi + 1) * 512]
                        if nco % 2 == 0:
                            last_copy["v"] = nc.vector.tensor_copy(dst, po[:])
                        else:
                            last_copy["s"] = nc.scalar.activation(
                                dst, po[:], mybir.ActivationFunctionType.Copy
                            )
                        nco += 1
                    pending.append((g, st, stg))
                    if g == 0 and st == 3:
                        emit_s_chain()
                    if len(pending) > 3:
                        flush_stage()
            while pending:
                flush_stage()

    nc.compile()
    return nc


_NC_CACHE = None
_LAST_IN_MAPS = None


def _get_nc():
    global _NC_CACHE
    if _NC_CACHE is None:
        _NC_CACHE = build_kernel()
    return _NC_CACHE


def _host_consts(x, Wconv, bconv):
    Tout = T - K + 1
    Ntot = B_TOTAL * Tout * F
    Wsum = Wconv.sum(axis=2).astype(np.float64)  # (K, C)
    head = x[:, : K - 1, :].astype(np.float64).sum(axis=0)  # (4, C)
    tail = x[:, T - (K - 1) :, :].astype(np.float64).sum(axis=0)  # (4, C)
    pre = np.concatenate([np.zeros((1, C)), np.cumsum(head, axis=0)], axis=0)
    suf = np.concatenate([np.zeros((1, C)), np.cumsum(tail[::-1], axis=0)], axis=0)
    edge = 0.0
    for k in range(K):
        edge += (Wsum[k] * (pre[k] + suf[K - 1 - k])).sum()
    qc = (Wsum.sum(axis=0) / Ntot).astype(np.float32)
    qcvec = np.tile(qc, 4).reshape(128, 1)
    c1 = np.float32(-edge / Ntot + float(np.mean(bconv)))
    return qcvec, np.array([[c1]], np.float32)


def _build_ab(Wconv):
    A = np.zeros((128, 128), np.float32)
    B = np.zeros((128, 128), np.float32)
    for sp in range(4):
        for so in range(4):
            k = sp - so
            if 0 <= k < K:
                A[sp * 32 : (sp + 1) * 32, so * 32 : (so + 1) * 32] = Wconv[k]
            k2 = sp - so + 4
            if 0 <= k2 < K:
                B[sp * 32 : (sp + 1) * 32, so * 32 : (so + 1) * 32] = Wconv[k2]
    return A.astype(np.float16), B.astype(np.float16)


def kernel(x, Wconv, bconv, Wdef):
    x = np.ascontiguousarray(np.asarray(x, np.float32))
    Wconv = np.asarray(Wconv, np.float32)
    bconv = np.asarray(bconv, np.float32)
    Wdef = np.asarray(Wdef, np.float32)

    nc = _get_nc()
    A, B = _build_ab(Wconv)
    qcvec, c1 = _host_consts(x, Wconv, bconv)
    ident = np.eye(128, dtype=np.float32)
    taps = (np.arange(K, dtype=np.float32) - (K // 2)).reshape(1, K)
    wdef_r = Wdef.reshape(1, K).astype(np.float32)

    in_maps = []
    for core in range(N_CORES):
        in_maps.append(
            {
                "x": x[core * BPC : (core + 1) * BPC],
                "wa": A,
                "wb": B,
                "ident": ident,
                "qcv": qcvec,
                "c1": c1,
                "taps": taps,
                "wdef": wdef_r,
            }
        )
    global _LAST_IN_MAPS
    _LAST_IN_MAPS = in_maps
    res = run_bass_kernel_spmd(nc, in_maps, list(range(N_CORES)))
    Tout = T - K + 1
    out = np.empty((B_TOTAL, Tout, F), np.float32)
    for core in range(N_CORES):
        o = res.results[core]["out"].reshape(BPC, QB * 4, F)
        out[core * BPC : (core + 1) * BPC] = o[:, :Tout, :]
    if np.any(bconv):
        out += bconv.reshape(1, 1, F)
    return out


# revision 27
# speedup vs baseline: 1.2404x; 1.0961x over previous
"""DeformableConv1D Trainium2 kernel.

Math: the reference reduces to
    offset = conv1d(x, Wconv) + bconv
    m = mean(offset);  scale_k = relu(1 - |m + R_k|);  s = sum_k Wdef[k]*scale_k
    out = conv1d(s*x, Wconv) + bconv = s * conv_nobias(x) + bconv

mean(offset) only needs per-channel sums of x (windowed; the tiny edge
corrections use 8 rows per batch and are folded into a host-precomputed
constant), so the device does:

  Phase 1 (streaming x once): for each 2MB load tile (natural layout, 4 rows
  per partition) run a fp32r ones-matmul (channel sums, accumulated in PSUM)
  and 4 PE transposes per psum group; copy each [128,512] psum group into the
  resident transposed tensor xt[(s',c), block] (cast to fp16).

  AllReduce 512B of partial sums across the 8 cores; compute the scalar s on
  device.

  Phase 2: polyphase conv: per 128-block output tile two fp16 matmuls
  (A-weights on the xt window, B-weights on the +1-shifted window) accumulate
  in PSUM; unscaled [128,512] copies drain PSUM into 1MB stages; each stage is
  scaled in place by s (the only op gated on the AllReduce) and DMA'd out.

Sharding: data-parallel over batch (2 batches per core x 8 cores).
bconv is all-zero in this problem; if not, it is added on the host.
"""

import numpy as np

import concourse.bacc as bacc
import concourse.bass as bass
import concourse.mybir as mybir
import concourse.tile as tile
from concourse.bass_utils import run_bass_kernel_spmd

FP = mybir.dt.float32
FPR = mybir.dt.float32r
CONV_DT = mybir.dt.float16

N_CORES = 8
B_TOTAL = 16
T = 65536
C = 32
F = 32
K = 5

BPC = B_TOTAL // N_CORES      # batches per core
U = 32                        # sub-tiles ([128,128] fp32) per load tile
NT = T // (4 * 128 * U)       # load tiles per batch (4)
QB = T // 4                   # blocks per batch (padded output length)
OT = QB // 128                # output tiles per batch (128)
GRP = U // 4                  # psum groups per load tile (8)
ST = 32                       # out-tiles per staged store (2 MiB)
NST = OT // ST                # stages per batch (8)
GPST = ST // 4                # psum groups per stage (4)


def build_kernel():
    nc = bacc.Bacc(
        "TRN2",
        target_bir_lowering=False,
        debug=False,
        enable_asserts=False,
        num_devices=N_CORES,
    )
    x = nc.dram_tensor("x", [BPC, T, C], FP, kind="ExternalInput").ap()
    wa = nc.dram_tensor("wa", [128, 128], CONV_DT, kind="ExternalInput").ap()
    wb = nc.dram_tensor("wb", [128, 128], CONV_DT, kind="ExternalInput").ap()
    ident = nc.dram_tensor("ident", [128, 128], FP, kind="ExternalInput").ap()
    qcv = nc.dram_tensor("qcv", [128, 1], FP, kind="ExternalInput").ap()
    c1 = nc.dram_tensor("c1", [1, 1], FP, kind="ExternalInput").ap()
    taps = nc.dram_tensor("taps", [1, K], FP, kind="ExternalInput").ap()
    wdef = nc.dram_tensor("wdef", [1, K], FP, kind="ExternalInput").ap()
    out = nc.dram_tensor("out", [BPC, QB, 128], FP, kind="ExternalOutput").ap()

    # load tiles: batch g, tile tt, partition p holds 4 units of 32 rows
    x_v = x.rearrange("g (tt v p r) c -> g tt p v (r c)", v=4, p=128, r=32)
    # out viewed for staged stores: block q = t*128 + p
    out_v = out.rearrange("g (t p) f -> g p t f", p=128)

    with tile.TileContext(nc) as tc:
        with (
            tc.tile_pool(name="res", bufs=1) as res_pool,
            tc.tile_pool(name="xload", bufs=3) as xload_pool,
            tc.tile_pool(name="stage", bufs=4) as stage_pool,
            tc.tile_pool(name="consts", bufs=1) as cpool,
            tc.tile_pool(name="pst", bufs=2, space="PSUM") as pst_pool,
            tc.tile_pool(name="ps", bufs=3, space="PSUM") as ps_pool,
            tc.tile_pool(name="psmisc", bufs=1, space="PSUM") as psmisc_pool,
            tc.tile_pool(name="dram", bufs=1, space="DRAM") as dram_pool,
        ):
            # resident transposed x: column g*QB + q holds x[g, 4q:4q+4, :]
            # flattened as (s', c) on partitions.
            xt = res_pool.tile([128, BPC * QB], CONV_DT)

            identity = cpool.tile([128, 128], FP)
            nc.gpsimd.dma_start(identity[:], ident[:])
            wa_t = cpool.tile([128, 128], CONV_DT)
            nc.gpsimd.dma_start(wa_t[:], wa[:])
            wb_t = cpool.tile([128, 128], CONV_DT)
            nc.gpsimd.dma_start(wb_t[:], wb[:])
            qcv_t = cpool.tile([128, 1], FP)
            nc.gpsimd.dma_start(qcv_t[:], qcv[:])
            c1_t = cpool.tile([1, 1], FP)
            nc.gpsimd.dma_start(c1_t[:], c1[:])
            taps_t = cpool.tile([1, K], FP)
            nc.gpsimd.dma_start(taps_t[:], taps[:])
            wdef_t = cpool.tile([1, K], FP)
            nc.gpsimd.dma_start(wdef_t[:], wdef[:])

            ones_row = cpool.tile([1, 128], FP)
            nc.vector.memset(ones_row[:], 1.0)

            acc = cpool.tile([128, BPC * NT * 4], FP)
            nc.vector.memset(acc[:], 0.0)
            psmisc = psmisc_pool.tile([128, 1], FP)

            # ---- Phase 1: load, channel sums, transpose into xt ----
            ncopy = 0
            for g in range(BPC):
                for tt in range(NT):
                    xtile = xload_pool.tile([128, U * 128], FP)
                    nc.sync.dma_start(
                        xtile.rearrange("p (v rc) -> p v rc", v=4), x_v[g, tt]
                    )
                    for v in range(4):
                        pt = pst_pool.tile([128, 1024], FP, name="pt")
                        for j in range(8):
                            nc.tensor.transpose(
                                pt[:, j * 128 : (j + 1) * 128],
                                xtile[:, v * 1024 + j * 128 : v * 1024 + (j + 1) * 128],
                                identity[:],
                            )
                        base = g * QB + (tt * 4 + v) * 1024
                        src_ap = pt.rearrange("i (j p) -> i p j", j=8, p=128)
                        if v == 3:
                            nc.vector.tensor_reduce(
                                acc[:, ncopy : ncopy + 1],
                                pt[:],
                                axis=mybir.AxisListType.X,
                                op=mybir.AluOpType.add,
                            )
                            nc.vector.tensor_copy(xt[:, base : base + 1024], src_ap)
                        else:
                            nc.scalar.activation(
                                xt[:, base : base + 1024],
                                src_ap,
                                mybir.ActivationFunctionType.Copy,
                                accum_out=acc[:, ncopy : ncopy + 1],
                            )
                        ncopy += 1

            # per-(s',c) partial sums
            localsum = cpool.tile([128, 1], FP)
            nc.vector.tensor_reduce(
                localsum[:],
                acc[:],
                axis=mybir.AxisListType.X,
                op=mybir.AluOpType.add,
            )
            # local dot: m_local = sum_p localsum[p] * qcvec[p]  (PE, ungated)
            nc.tensor.matmul(psmisc[0:1, 0:1], localsum[:], qcv_t[:])
            mloc = cpool.tile([1, 1], FP)
            nc.vector.tensor_copy(mloc[:], psmisc[0:1, 0:1])

            # ---- AllReduce the scalar across cores ----
            ar_in = dram_pool.tile([1, 1], FP)
            ar_out = dram_pool.tile([1, 1], FP, addr_space="Shared")
            nc.gpsimd.dma_start(ar_in[:], mloc[:])
            nc.gpsimd.collective_compute(
                "AllReduce",
                mybir.AluOpType.add,
                replica_groups=[list(range(N_CORES))],
                ins=[ar_in.opt()],
                outs=[ar_out.opt()],
            )
            mg = cpool.tile([1, 1], FP)
            nc.sync.dma_start(mg[:], ar_out[:])

            # ---- s computation, deferred: emitted after stage-2 copies so the
            # DVE/ACT instruction streams don't stall on the AllReduce before
            # draining early phase-2 PSUM groups ----
            s_b = cpool.tile([128, 1], FP)

            def emit_s_chain():
                m1 = cpool.tile([1, 1], FP)
                nc.vector.tensor_tensor(
                    m1[:], mg[:], c1_t[:], op=mybir.AluOpType.add
                )
                t1 = cpool.tile([1, K], FP)
                nc.vector.tensor_scalar_add(t1[:], taps_t[:], m1[:])
                t2 = cpool.tile([1, K], FP)
                nc.scalar.activation(t2[:], t1[:], mybir.ActivationFunctionType.Abs)
                t3 = cpool.tile([1, K], FP)
                nc.vector.tensor_scalar(
                    t3[:],
                    t2[:],
                    -1.0,
                    1.0,
                    op0=mybir.AluOpType.mult,
                    op1=mybir.AluOpType.add,
                )
                t4 = cpool.tile([1, K], FP)
                nc.vector.tensor_scalar_max(t4[:], t3[:], 0.0)
                t5 = cpool.tile([1, K], FP)
                nc.vector.tensor_tensor(
                    t5[:], t4[:], wdef_t[:], op=mybir.AluOpType.mult
                )
                s11 = cpool.tile([1, 1], FP)
                nc.vector.tensor_reduce(
                    s11[:], t5[:], axis=mybir.AxisListType.X, op=mybir.AluOpType.add
                )
                nc.gpsimd.partition_broadcast(s_b[:], s11[:])

            # ---- Phase 2: polyphase conv ----
            # scales (the only ops gated on the AllReduce) and stores trail the
            # copies by 2 stages so engines keep draining PSUM during the wait
            nco = 0
            pending = []

            def flush_stage():
                g0, st0, stg0 = pending.pop(0)
                if (g0 * NST + st0) % 2 == 0:
                    nc.vector.tensor_scalar_mul(stg0[:], stg0[:], s_b[:])
                else:
                    nc.scalar.activation(
                        stg0[:],
                        stg0[:],
                        mybir.ActivationFunctionType.Copy,
                        scale=s_b[:],
                    )
                nc.sync.dma_start(
                    out_v[g0, :, st0 * ST : (st0 + 1) * ST, :],
                    stg0.rearrange("p (t f) -> p t f", t=ST),
                )

            for g in range(BPC):
                for st in range(NST):
                    stg = stage_pool.tile([128, ST * 128], FP)
                    for gi in range(GPST):
                        po = ps_pool.tile([128, 512], FP, name="ps")
                        for i in range(4):
                            ot = st * ST + gi * 4 + i
                            col0 = g * QB + 128 * ot
                            sl = po[:, i * 128 : (i + 1) * 128]
                            nc.tensor.matmul(
                                sl, xt[:, col0 : col0 + 128], wa_t[:],
                                start=True, stop=False,
                            )
                            mb = 127 if (g == BPC - 1 and ot == OT - 1) else 128
                            nc.tensor.matmul(
                                po[0:mb, i * 128 : (i + 1) * 128],
                                xt[:, col0 + 1 : col0 + 1 + mb], wb_t[:],
                                start=False, stop=True,
                            )
                        dst = stg[:, gi * 512 : (gi + 1) * 512]
                        if nco % 2 == 0:
                            nc.vector.tensor_copy(dst, po[:])
                        else:
                            nc.scalar.activation(
                                dst, po[:], mybir.ActivationFunctionType.Copy
                            )
                        nco += 1
                    pending.append((g, st, stg))
                    if g == 0 and st == 2:
                        emit_s_chain()
                    if len(pending) > 3:
                        flush_stage()
            while pending:
                flush_stage()

    nc.compile()
    return nc


_NC_CACHE = None
_LAST_IN_MAPS = None


def _get_nc():
    global _NC_CACHE
    if _NC_CACHE is None:
        _NC_CACHE = build_kernel()
    return _NC_CACHE


def _host_consts(x, Wconv, bconv):
    Tout = T - K + 1
    Ntot = B_TOTAL * Tout * F
    Wsum = Wconv.sum(axis=2).astype(np.float64)  # (K, C)
    head = x[:, : K - 1, :].astype(np.float64).sum(axis=0)  # (4, C)
    tail = x[:, T - (K - 1) :, :].astype(np.float64).sum(axis=0)  # (4, C)
    pre = np.concatenate([np.zeros((1, C)), np.cumsum(head, axis=0)], axis=0)
    suf = np.concatenate([np.zeros((1, C)), np.cumsum(tail[::-1], axis=0)], axis=0)
    edge = 0.0
    for k in range(K):
        edge += (Wsum[k] * (pre[k] + suf[K - 1 - k])).sum()
    qc = (Wsum.sum(axis=0) / Ntot).astype(np.float32)
    qcvec = np.tile(qc, 4).reshape(128, 1)
    c1 = np.float32(-edge / Ntot + float(np.mean(bconv)))
    return qcvec, np.array([[c1]], np.float32)


def _build_ab(Wconv):
    A = np.zeros((128, 128), np.float32)
    B = np.zeros((128, 128), np.float32)
    for sp in range(4):
        for so in range(4):
            k = sp - so
            if 0 <= k < K:
                A[sp * 32 : (sp + 1) * 32, so * 32 : (so + 1) * 32] = Wconv[k]
            k2 = sp - so + 4
            if 0 <= k2 < K:
                B[sp * 32 : (sp + 1) * 32, so * 32 : (so + 1) * 32] = Wconv[k2]
    return A.astype(np.float16), B.astype(np.float16)


def kernel(x, Wconv, bconv, Wdef):
    x = np.ascontiguousarray(np.asarray(x, np.float32))
    Wconv = np.asarray(Wconv, np.float32)
    bconv = np.asarray(bconv, np.float32)
    Wdef = np.asarray(Wdef, np.float32)

    nc = _get_nc()
    A, B = _build_ab(Wconv)
    qcvec, c1 = _host_consts(x, Wconv, bconv)
    ident = np.eye(128, dtype=np.float32)
    taps = (np.arange(K, dtype=np.float32) - (K // 2)).reshape(1, K)
    wdef_r = Wdef.reshape(1, K).astype(np.float32)

    in_maps = []
    for core in range(N_CORES):
        in_maps.append(
            {
                "x": x[core * BPC : (core + 1) * BPC],
                "wa": A,
                "wb": B,
                "ident": ident,
                "qcv": qcvec,
                "c1": c1,
                "taps": taps,
                "wdef": wdef_r,
            }
        )
    global _LAST_IN_MAPS
    _LAST_IN_MAPS = in_maps
    res = run_bass_kernel_spmd(nc, in_maps, list(range(N_CORES)))
    Tout = T - K + 1
    out = np.empty((B_TOTAL, Tout, F), np.float32)
    for core in range(N_CORES):
        o = res.results[core]["out"].reshape(BPC, QB * 4, F)
        out[core * BPC : (core + 1) * BPC] = o[:, :Tout, :]
    if np.any(bconv):
        out += bconv.reshape(1, 1, F)
    return out
